# revision 2
# baseline (speedup 1.0000x reference)
"""Cross-attention (S2Audio) Trainium2 Bass kernel.

Sharding: data-parallel over the clip batch B=8 -> one batch element per
NeuronCore.  Per core the kernel computes, for its batch element b:

  q = (audio_patch + pos_a) @ q_w.T + q_b          (1568, 768)
  k,v = (s_x_patch + pos_s) @ kv_w.T + kv_b        (1568, 768) each
  out = softmax(q k^T / sqrt(64)) v  per 12 heads  -> proj -> (1568, 768)

Host prep is layout/elementwise only: weight transposes, positional-embedding
combine + add (O(N*D)), bf16 casts, sharding slices.  All matmuls/softmax run
on device.

On-device layout/dtype strategy:
  * matmul operands are bf16 (PE runs fp32 matmuls at 4 cycles/row vs 1 for
    bf16); every accumulation is fp32 in PSUM, softmax statistics fp32.
  * activations arrive feature-major (host-transposed) as x_feat [768, tok].
  * K projection produces feature-major k_feat [768, 1568] (lhsT = W^T chunk,
    rhs = x_feat) so heads live on partitions (contraction dim of the scores
    matmul).  Q is produced the same way per 512-token block, just in time
    inside the attention loop.
  * V projection produces token-major v [1568, 768] (lhsT = x_feat chunk,
    rhs = W^T), stored interleaved [128, 12, 65] with a ones-column per head.
  * scores are computed TRANSPOSED: sT[nk, nq] = k_feat_h(chunk)^T @ q_feat_h,
    exp() applied on ScalarE straight out of PSUM with the 1/sqrt(64) scale
    fused, output bf16.  No max-subtraction (scores are O(+-6); exp is safe in
    fp32 and matches the reference softmax mathematically).
  * PV: out_aug[65, nq] = v_aug^T @ exp_sT accumulated over nk chunks; row 64
    (from the ones column) is the softmax denominator.  Normalization:
    DVE reciprocal (fp32) + K=1 fp32 broadcast matmul + DVE multiply.
  * O-projection back to token-major fp32, then DMA out.
"""

import numpy as np
from contextlib import ExitStack

B, T, NPATCH, APATCH, D, H = 8, 8, 196, 196, 768, 12
HD = D // H                      # 64
SCALE = float(HD) ** -0.5        # 0.125
NT = NPATCH * T                  # 1568 tokens (same count for q and kv side)
P = 128
DC = D // P                      # 6 feature chunks
N_CORES = 8

# token chunks (partition-dim tiling): 12 x 128 + 1 x 32
TOK_CHUNKS = [(i * P, min(P, NT - i * P)) for i in range((NT + P - 1) // P)]
# nq blocks for the attention/output stage
NQB = 512
NQ_BLOCKS = [(s, min(NQB, NT - s)) for s in range(0, NT, NQB)]

_CACHE: dict = {}


def _build_nc(qb_nz: bool, kb_nz: bool, vb_nz: bool, pb_nz: bool):
    import concourse.mybir as mybir
    from concourse import bacc
    from concourse.tile import TileContext

    f32 = mybir.dt.float32
    bf16 = mybir.dt.bfloat16
    AF = mybir.ActivationFunctionType

    nc = bacc.Bacc("TRN2", target_bir_lowering=False, debug=False,
                   num_devices=N_CORES)

    xsT = nc.dram_tensor("xsT", [D, NT], bf16, kind="ExternalInput")
    xaT = nc.dram_tensor("xaT", [D, NT], bf16, kind="ExternalInput")
    qwT = nc.dram_tensor("qwT", [D, D], bf16, kind="ExternalInput")
    kvwT = nc.dram_tensor("kvwT", [D, 2 * D], bf16, kind="ExternalInput")
    projT = nc.dram_tensor("projT", [D, D], bf16, kind="ExternalInput")
    qb = nc.dram_tensor("qb", [P, DC], f32, kind="ExternalInput") if qb_nz else None
    kb = nc.dram_tensor("kb", [P, DC], f32, kind="ExternalInput") if kb_nz else None
    vb = nc.dram_tensor("vb", [1, D], bf16, kind="ExternalInput") if vb_nz else None
    pb = nc.dram_tensor("pb", [1, D], bf16, kind="ExternalInput") if pb_nz else None
    out = nc.dram_tensor("out", [NT, D], f32, kind="ExternalOutput")

    with TileContext(nc) as tc, ExitStack() as ctx:
        consts = ctx.enter_context(tc.tile_pool(name="consts", bufs=1))
        persist = ctx.enter_context(tc.tile_pool(name="persist", bufs=1))

        ones_bf = consts.tile([1, P], bf16, tag="ones_bf")
        nc.gpsimd.memset(ones_bf[:], 1.0)
        ones_f32 = consts.tile([1, HD], f32, tag="ones_f32")
        nc.gpsimd.memset(ones_f32[:], 1.0)
        qb_sb = kb_sb = vb_sb = pb_sb = None
        if qb_nz:
            qb_sb = consts.tile([P, DC], f32, tag="qb")
            nc.sync.dma_start(qb_sb[:], qb[:])
        if kb_nz:
            kb_sb = consts.tile([P, DC], f32, tag="kb")
            nc.sync.dma_start(kb_sb[:], kb[:])
        if vb_nz:
            vb_sb = consts.tile([1, D], bf16, tag="vb")
            nc.sync.dma_start(vb_sb[:], vb[:])
        if pb_nz:
            pb_sb = consts.tile([1, D], bf16, tag="pb")
            nc.sync.dma_start(pb_sb[:], pb[:])

        # persistent SBUF tensors: K (feature-major) and V (token-major)
        k_feat = [persist.tile([P, NT], bf16, tag=f"k_feat{c}", name=f"k_feat{c}")
                  for c in range(DC)]
        v_st = [persist.tile([P, H, HD + 1], bf16, tag=f"v{i}", name=f"v{i}")
                for i in range(len(TOK_CHUNKS))]

        # ---------------- phase 1: K and V projections ----------------
        with ExitStack() as ph:
            wtp = ph.enter_context(tc.tile_pool(name="wtp", bufs=1))
            xfp = ph.enter_context(tc.tile_pool(name="xfp", bufs=1))
            ps1 = ph.enter_context(tc.tile_pool(name="ps1", bufs=6, space="PSUM"))

            kvw_sb = wtp.tile([P, DC, 2 * D], bf16, tag="kvw", name="kvw")
            nc.sync.dma_start(kvw_sb[:], kvwT.rearrange("(c p) d -> p c d", p=P))

            xs_feat = [xfp.tile([P, NT], bf16, tag=f"xsf{c}", name=f"xsf{c}")
                       for c in range(DC)]
            for c in range(DC):
                nc.sync.dma_start(xs_feat[c][:], xsT[c * P:(c + 1) * P, :])

            # K projection (feature-major)
            for m in range(DC):
                for (n0, nw) in NQ_BLOCKS:
                    ps = ps1.tile([P, NQB], f32, tag="big", name="kproj")
                    for c in range(DC):
                        nc.tensor.matmul(ps[:, :nw],
                                         kvw_sb[:, c, m * P:(m + 1) * P],
                                         xs_feat[c][:, n0:n0 + nw],
                                         start=(c == 0), stop=(c == DC - 1))
                    if kb_nz:
                        nc.scalar.activation(k_feat[m][:, n0:n0 + nw],
                                             ps[:, :nw], AF.Identity,
                                             bias=kb_sb[:, m:m + 1])
                    else:
                        nc.vector.tensor_copy(k_feat[m][:, n0:n0 + nw],
                                              ps[:, :nw])

            # V projection (token-major, interleaved with ones column)
            for ti, (t0, tw) in enumerate(TOK_CHUNKS):
                for half in range(2):
                    ps = ps1.tile([P, NQB], f32, tag="big", name="vproj")
                    for c in range(DC):
                        nc.tensor.matmul(
                            ps[:tw, :384],
                            xs_feat[c][:, t0:t0 + tw],
                            kvw_sb[:, c, D + half * 384:D + (half + 1) * 384],
                            start=(c == 0), stop=(c == DC - 1 and not vb_nz))
                    if vb_nz:
                        nc.tensor.matmul(
                            ps[:tw, :384], ones_bf[:, :tw],
                            vb_sb[:, half * 384:(half + 1) * 384],
                            start=False, stop=True)
                    nc.vector.tensor_copy(
                        v_st[ti][:tw, half * 6:(half + 1) * 6, :HD],
                        ps[:tw, :384].rearrange("p (h d) -> p h d", d=HD))
                nc.vector.memset(v_st[ti][:tw, :, HD:], 1.0)

        # -------- phase 2: per-block Q proj + attention + O-proj --------
        with ExitStack() as ph:
            qwp = ph.enter_context(tc.tile_pool(name="qwp", bufs=1))
            pwp = ph.enter_context(tc.tile_pool(name="pwp", bufs=1))
            xfb = ph.enter_context(tc.tile_pool(name="xfb", bufs=2))
            qfb = ph.enter_context(tc.tile_pool(name="qfb", bufs=2))
            expp = ph.enter_context(tc.tile_pool(name="expp", bufs=15))
            ofp = ph.enter_context(tc.tile_pool(name="ofp", bufs=2))
            otp = ph.enter_context(tc.tile_pool(name="otp", bufs=2))
            nrm = ph.enter_context(tc.tile_pool(name="nrm", bufs=3))
            ps2 = ph.enter_context(tc.tile_pool(name="ps2", bufs=4, space="PSUM"))
            pvps = ph.enter_context(tc.tile_pool(name="pvps", bufs=2, space="PSUM"))
            bcps = ph.enter_context(tc.tile_pool(name="bcps", bufs=2, space="PSUM"))

            qw_sb = qwp.tile([P, DC, D], bf16, tag="qw", name="qw")
            nc.sync.dma_start(qw_sb[:], qwT.rearrange("(c p) d -> p c d", p=P))
            pw_sb = pwp.tile([P, DC, D], bf16, tag="pw", name="pw")
            nc.sync.dma_start(pw_sb[:], projT.rearrange("(c p) d -> p c d", p=P))

            for (n0, nw) in NQ_BLOCKS:
                # Q for this block: load (already feature-major) + project
                xa_feat = [xfb.tile([P, NQB], bf16, tag=f"xaf{c}", name=f"xaf{c}")
                           for c in range(DC)]
                for c in range(DC):
                    nc.sync.dma_start(xa_feat[c][:, :nw],
                                      xaT[c * P:(c + 1) * P, n0:n0 + nw])
                q_feat = [qfb.tile([P, NQB], bf16, tag=f"qf{c}", name=f"qf{c}")
                          for c in range(DC)]
                for m in range(DC):
                    ps = ps2.tile([P, NQB], f32, tag="big", name="qproj")
                    for c in range(DC):
                        nc.tensor.matmul(ps[:, :nw],
                                         qw_sb[:, c, m * P:(m + 1) * P],
                                         xa_feat[c][:, :nw],
                                         start=(c == 0), stop=(c == DC - 1))
                    if qb_nz:
                        nc.scalar.activation(q_feat[m][:, :nw], ps[:, :nw],
                                             AF.Identity, bias=qb_sb[:, m:m + 1])
                    else:
                        nc.vector.tensor_copy(q_feat[m][:, :nw], ps[:, :nw])

                out_feat = [ofp.tile([P, NQB], bf16, tag=f"of{c}", name=f"of{c}")
                            for c in range(DC)]
                for h in range(H):
                    hc, hp = h // 2, (h % 2) * HD
                    exp_tiles = []
                    # scores^T chunks + fused scale+exp (bf16 out)
                    for ti, (t0, tw) in enumerate(TOK_CHUNKS):
                        ps = ps2.tile([P, NQB], f32, tag="big", name="score")
                        nc.tensor.matmul(
                            ps[:tw, :nw],
                            k_feat[hc][hp:hp + HD, t0:t0 + tw],
                            q_feat[hc][hp:hp + HD, :nw],
                            start=True, stop=True)
                        et = expp.tile([P, NQB], bf16, tag="exp", name="exp")
                        nc.scalar.activation(et[:tw, :nw], ps[:tw, :nw],
                                             AF.Exp, scale=SCALE)
                        exp_tiles.append(et)
                    # PV with ones-row -> [65, nw]; row 64 = denominator
                    pv = pvps.tile([HD + 1, NQB], f32, tag="pv", name="pv")
                    for ti, (t0, tw) in enumerate(TOK_CHUNKS):
                        nc.tensor.matmul(pv[:, :nw],
                                         v_st[ti][:tw, h, :],
                                         exp_tiles[ti][:tw, :nw],
                                         start=(ti == 0),
                                         stop=(ti == len(TOK_CHUNKS) - 1))
                    # normalize: fp32 reciprocal of denom, broadcast over
                    # partitions via a K=1 fp32 matmul, then one DVE mul
                    rec = nrm.tile([1, NQB], f32, tag="rec", name="rec")
                    nc.vector.reciprocal(rec[:, :nw], pv[HD:HD + 1, :nw])
                    bc = bcps.tile([HD, NQB], f32, tag="bc", name="bc")
                    nc.tensor.matmul(bc[:, :nw], ones_f32[:1, :HD],
                                     rec[:, :nw], start=True, stop=True)
                    bcs = nrm.tile([HD, NQB], f32, tag="bcs", name="bcs")
                    nc.vector.tensor_copy(bcs[:, :nw], bc[:, :nw])
                    nc.vector.tensor_mul(out_feat[hc][hp:hp + HD, :nw],
                                         pv[:HD, :nw], bcs[:, :nw])

                # O-projection for this block (token-major out)
                for (c0, cw) in [(c, min(P, nw - c)) for c in range(0, nw, P)]:
                    ot = otp.tile([P, D], f32, tag="ot", name="ot")
                    for half in range(2):
                        ps = ps2.tile([P, NQB], f32, tag="big", name="oproj")
                        for c in range(DC):
                            nc.tensor.matmul(
                                ps[:cw, :384],
                                out_feat[c][:, c0:c0 + cw],
                                pw_sb[:, c, half * 384:(half + 1) * 384],
                                start=(c == 0), stop=(c == DC - 1 and not pb_nz))
                        if pb_nz:
                            nc.tensor.matmul(
                                ps[:cw, :384], ones_bf[:, :cw],
                                pb_sb[:, half * 384:(half + 1) * 384],
                                start=False, stop=True)
                        nc.vector.tensor_copy(
                            ot[:cw, half * 384:(half + 1) * 384], ps[:cw, :384])
                    nc.sync.dma_start(out[n0 + c0:n0 + c0 + cw, :], ot[:cw, :])

    nc.finalize()
    return nc


def kernel(**inputs) -> np.ndarray:
    import ml_dtypes
    bf = ml_dtypes.bfloat16

    s_x = np.asarray(inputs["s_x"], np.float32)
    audio = np.asarray(inputs["audio"], np.float32)
    q_w = np.asarray(inputs["q_w"], np.float32)
    q_b = np.asarray(inputs["q_b"], np.float32)
    kv_w = np.asarray(inputs["kv_w"], np.float32)
    kv_b = np.asarray(inputs["kv_b"], np.float32)
    proj_w = np.asarray(inputs["proj_w"], np.float32)
    proj_b = np.asarray(inputs["proj_b"], np.float32)

    # host prep: layout + O(N*D) positional add + bf16 casts only
    pos_s = (np.asarray(inputs["clip_space_pos"], np.float32)[:, None, :]
             + np.asarray(inputs["clip_temporal_pos"], np.float32)[None, :, :]
             ).reshape(NT, D)
    pos_a = (np.asarray(inputs["audio_space_pos"], np.float32)[:, None, :]
             + np.asarray(inputs["audio_temporal_pos"], np.float32)[None, :, :]
             ).reshape(NT, D)
    qwT = np.ascontiguousarray(q_w.T).astype(bf)
    kvwT = np.ascontiguousarray(kv_w.T).astype(bf)
    projT = np.ascontiguousarray(proj_w.T).astype(bf)
    qb_nz = bool(np.any(q_b))
    kb_nz = bool(np.any(kv_b[:D]))
    vb_nz = bool(np.any(kv_b[D:]))
    pb_nz = bool(np.any(proj_b))

    key = (qb_nz, kb_nz, vb_nz, pb_nz)
    if key not in _CACHE:
        _CACHE[key] = _build_nc(*key)
    nc = _CACHE[key]

    shared = {"qwT": qwT, "kvwT": kvwT, "projT": projT}
    if qb_nz:
        shared["qb"] = np.ascontiguousarray(q_b.reshape(DC, P).T)
    if kb_nz:
        shared["kb"] = np.ascontiguousarray(kv_b[:D].reshape(DC, P).T)
    if vb_nz:
        shared["vb"] = np.ascontiguousarray(kv_b[D:].reshape(1, D)).astype(bf)
    if pb_nz:
        shared["pb"] = np.ascontiguousarray(proj_b.reshape(1, D)).astype(bf)

    in_maps = []
    for b in range(N_CORES):
        m = dict(shared)
        m["xsT"] = np.ascontiguousarray(
            (s_x[1:, b * T:(b + 1) * T, :].reshape(NT, D) + pos_s).T).astype(bf)
        m["xaT"] = np.ascontiguousarray(
            (audio[2:, b * T:(b + 1) * T, :].reshape(NT, D) + pos_a).T).astype(bf)
        in_maps.append(m)

    from concourse.bass_utils import run_bass_kernel_spmd
    res = run_bass_kernel_spmd(nc, in_maps, core_ids=list(range(N_CORES)))
    globals()["_LAST_RESULT"] = res

    out_full = np.empty((2 + APATCH, B * T, D), np.float32)
    out_full[:2] = audio[:2]
    for b in range(N_CORES):
        out_full[2:, b * T:(b + 1) * T, :] = \
            res.results[b]["out"].reshape(APATCH, T, D)
    return out_full



# revision 14
# speedup vs baseline: 1.3006x; 1.3006x over previous
"""Cross-attention (S2Audio) Trainium2 Bass kernel.

Sharding: data-parallel over the clip batch B=8 -> one batch element per
NeuronCore.  Per core the kernel computes, for its batch element b:

  q = (audio_patch + pos_a) @ q_w.T + q_b          (1568, 768)
  k,v = (s_x_patch + pos_s) @ kv_w.T + kv_b        (1568, 768) each
  out = softmax(q k^T / sqrt(64)) v  per 12 heads  -> proj -> (1568, 768)

Perf design notes (v2):
  * TRN2 PE HAM clock gate: the PE idles at 1.2 GHz and only reaches
    2.4 GHz after ~3.4us of *continuous* matmul activity; any multi-us
    idle re-throttles.  v1 stalled the PE on a DVE reciprocal + K=1
    broadcast matmul per (head, block) -> the whole attention phase ran
    cold at half clock.  v2 keeps the PE stream gap-free:
      - softmax normalization runs entirely on DVE/GpSimd
        (batched reciprocal + gpsimd partition_broadcast), never on PE.
      - head-pipelined attention: scores of head h interleave with PV of
        head h-1 chunk by chunk, matching ScalarE exp pace.
      - O-projection of block b-1 interleaves with head-0 scores of
        block b; Q-projection of block b+1 interleaves with the last
        head's PV -> no idle at block boundaries.
  * 1024-wide moving operands (bf16 max) halve instruction counts;
    PSUM tiles span 2 banks.
  * matmul operands bf16, accumulation fp32 in PSUM, softmax stats fp32.
  * scores computed transposed sT[nk, nq] = k_feat_h^T @ q_feat_h, exp on
    ScalarE straight out of PSUM with the 1/sqrt(64) scale fused, bf16
    out.  No max-subtraction (scores are O(+-6); exp is safe in fp32).
  * PV: out_aug[65, nq] = v_aug^T @ exp_sT over nk chunks; row 64 (ones
    column) is the softmax denominator.
"""

import numpy as np
from contextlib import ExitStack

B, T, NPATCH, APATCH, D, H = 8, 8, 196, 196, 768, 12
HD = D // H                      # 64
SCALE = float(HD) ** -0.5        # 0.125
NT = NPATCH * T                  # 1568 tokens (same count for q and kv side)
P = 128
DC = D // P                      # 6 feature chunks
N_CORES = 8

# token chunks (partition-dim tiling): 12 x 128 + 1 x 32
TOK_CHUNKS = [(i * P, min(P, NT - i * P)) for i in range((NT + P - 1) // P)]
NC_CH = len(TOK_CHUNKS)          # 13
# nq blocks for the attention/output stage (bf16 moving operand max 1024)
NQB = 1024
NQ_BLOCKS = [(s, min(NQB, NT - s)) for s in range(0, NT, NQB)]  # 1024, 544


def _subs(nw, step=512):
    """Split a free-dim width into matmul-legal (<=512) sub-blocks."""
    return [(s, min(step, nw - s)) for s in range(0, nw, step)]

_CACHE: dict = {}


def _build_nc(qb_nz: bool, kb_nz: bool, vb_nz: bool, pb_nz: bool):
    import concourse.mybir as mybir
    from concourse import bacc
    from concourse.tile import TileContext

    f32 = mybir.dt.float32
    bf16 = mybir.dt.bfloat16
    AF = mybir.ActivationFunctionType

    nc = bacc.Bacc("TRN2", target_bir_lowering=False, debug=False,
                   num_devices=N_CORES)

    xsT = nc.dram_tensor("xsT", [D, NT], bf16, kind="ExternalInput")
    xaT = nc.dram_tensor("xaT", [D, NT], bf16, kind="ExternalInput")
    qwT = nc.dram_tensor("qwT", [D, D], bf16, kind="ExternalInput")
    kvwT = nc.dram_tensor("kvwT", [D, 2 * D], bf16, kind="ExternalInput")
    projT = nc.dram_tensor("projT", [D, D], bf16, kind="ExternalInput")
    qb = nc.dram_tensor("qb", [P, DC], f32, kind="ExternalInput") if qb_nz else None
    kb = nc.dram_tensor("kb", [P, DC], f32, kind="ExternalInput") if kb_nz else None
    vb = nc.dram_tensor("vb", [1, D], bf16, kind="ExternalInput") if vb_nz else None
    pb = nc.dram_tensor("pb", [1, D], bf16, kind="ExternalInput") if pb_nz else None
    out = nc.dram_tensor("out", [NT, D], f32, kind="ExternalOutput")

    with TileContext(nc) as tc, ExitStack() as ctx:
        consts = ctx.enter_context(tc.tile_pool(name="consts", bufs=1))
        persist = ctx.enter_context(tc.tile_pool(name="persist", bufs=1))

        ones_bf = consts.tile([1, P], bf16, tag="ones_bf")
        nc.gpsimd.memset(ones_bf[:], 1.0)
        qb_sb = kb_sb = vb_sb = pb_sb = None
        if qb_nz:
            qb_sb = consts.tile([P, DC], f32, tag="qb")
            nc.sync.dma_start(qb_sb[:], qb[:])
        if kb_nz:
            kb_sb = consts.tile([P, DC], f32, tag="kb")
            nc.sync.dma_start(kb_sb[:], kb[:])
        if vb_nz:
            vb_sb = consts.tile([1, D], bf16, tag="vb")
            nc.sync.dma_start(vb_sb[:], vb[:])
        if pb_nz:
            pb_sb = consts.tile([1, D], bf16, tag="pb")
            nc.sync.dma_start(pb_sb[:], pb[:])

        # persistent SBUF tensors: K (feature-major) and V (token-major)
        k_feat = [persist.tile([P, NT], bf16, tag=f"k_feat{c}", name=f"k_feat{c}")
                  for c in range(DC)]
        v_st = [persist.tile([P, H, HD + 1], bf16, tag=f"v{i}", name=f"v{i}")
                for i in range(NC_CH)]

        # ---------------- phase 1: K and V projections ----------------
        with ExitStack() as ph:
            wtp = ph.enter_context(tc.tile_pool(name="wtp", bufs=1))
            xfp = ph.enter_context(tc.tile_pool(name="xfp", bufs=1))
            kps = ph.enter_context(tc.tile_pool(name="kps", bufs=2, space="PSUM"))
            vps = ph.enter_context(tc.tile_pool(name="vps", bufs=2, space="PSUM"))

            kvw_sb = wtp.tile([P, DC, 2 * D], bf16, tag="kvw", name="kvw")
            nc.sync.dma_start(kvw_sb[:], kvwT.rearrange("(c p) d -> p c d", p=P))

            xs_feat = [xfp.tile([P, NT], bf16, tag=f"xsf{c}", name=f"xsf{c}")
                       for c in range(DC)]
            for c in range(DC):
                nc.sync.dma_start(xs_feat[c][:], xsT[c * P:(c + 1) * P, :])

            # K projection (feature-major), 1024-wide streams
            for m in range(DC):
                for (n0, nw) in NQ_BLOCKS:
                    ps = kps.tile([P, NQB], f32, tag="kp", name="kproj")
                    for (s0, sw) in _subs(nw):
                        for c in range(DC):
                            nc.tensor.matmul(
                                ps[:, s0:s0 + sw],
                                kvw_sb[:, c, m * P:(m + 1) * P],
                                xs_feat[c][:, n0 + s0:n0 + s0 + sw],
                                start=(c == 0), stop=(c == DC - 1))
                    if kb_nz:
                        for (s0, sw) in _subs(nw):
                            nc.scalar.activation(
                                k_feat[m][:, n0 + s0:n0 + s0 + sw],
                                ps[:, s0:s0 + sw], AF.Identity,
                                bias=kb_sb[:, m:m + 1])
                    else:
                        for (s0, sw) in _subs(nw):
                            nc.vector.tensor_copy(
                                k_feat[m][:, n0 + s0:n0 + s0 + sw],
                                ps[:, s0:s0 + sw])

            # V projection (token-major, N=768, interleaved with ones col)
            for ti, (t0, tw) in enumerate(TOK_CHUNKS):
                ps = vps.tile([P, D], f32, tag="vp", name="vproj")
                for (s0, sw) in _subs(D):
                    for c in range(DC):
                        nc.tensor.matmul(
                            ps[:tw, s0:s0 + sw],
                            xs_feat[c][:, t0:t0 + tw],
                            kvw_sb[:, c, D + s0:D + s0 + sw],
                            start=(c == 0), stop=(c == DC - 1 and not vb_nz))
                    if vb_nz:
                        nc.tensor.matmul(
                            ps[:tw, s0:s0 + sw], ones_bf[:, :tw],
                            vb_sb[:, s0:s0 + sw],
                            start=False, stop=True)
                nc.vector.tensor_copy(
                    v_st[ti][:tw, :8, :HD],
                    ps[:tw, :512].rearrange("p (h d) -> p h d", d=HD))
                nc.vector.tensor_copy(
                    v_st[ti][:tw, 8:, :HD],
                    ps[:tw, 512:].rearrange("p (h d) -> p h d", d=HD))
                nc.vector.memset(v_st[ti][:tw, :, HD:], 1.0)

        # -------- phase 2: per-block Q proj + attention + O-proj --------
        with ExitStack() as ph:
            qwp = ph.enter_context(tc.tile_pool(name="qwp", bufs=1))
            pwp = ph.enter_context(tc.tile_pool(name="pwp", bufs=1))
            xfb = ph.enter_context(tc.tile_pool(name="xfb", bufs=2))
            qfb = ph.enter_context(tc.tile_pool(name="qfb", bufs=2))
            expp = ph.enter_context(tc.tile_pool(name="expp", bufs=NC_CH + 1))
            ofp = ph.enter_context(tc.tile_pool(name="ofp", bufs=1))
            otp = ph.enter_context(tc.tile_pool(name="otp", bufs=2))
            nrm = ph.enter_context(tc.tile_pool(name="nrm", bufs=2))
            rbcp = ph.enter_context(tc.tile_pool(name="rbcp", bufs=2))
            # one shared matmul-out pool (scores / q-proj / o-proj) + PV pool
            ps_mm = ph.enter_context(tc.tile_pool(name="ps_mm", bufs=2, space="PSUM"))
            ps_pv = ph.enter_context(tc.tile_pool(name="ps_pv", bufs=2, space="PSUM"))

            qw_sb = qwp.tile([P, DC, D], bf16, tag="qw", name="qw")
            nc.sync.dma_start(qw_sb[:], qwT.rearrange("(c p) d -> p c d", p=P))
            pw_sb = pwp.tile([P, DC, D], bf16, tag="pw", name="pw")
            nc.sync.dma_start(pw_sb[:], projT.rearrange("(c p) d -> p c d", p=P))

            n_blk = len(NQ_BLOCKS)

            def load_xa(b):
                n0, nw = NQ_BLOCKS[b]
                tiles = [xfb.tile([P, NQB], bf16, tag=f"xaf{c}", name=f"xaf{c}")
                         for c in range(DC)]
                for c in range(DC):
                    nc.sync.dma_start(tiles[c][:, :nw],
                                      xaT[c * P:(c + 1) * P, n0:n0 + nw])
                return tiles

            def qproj(b, xa_tiles):
                """Q projection of block b: DC groups of DC accumulating
                matmuls (N=nw up to 1024) + a DVE copy each."""
                n0, nw = NQ_BLOCKS[b]
                q_tiles = [qfb.tile([P, NQB], bf16, tag=f"qf{c}", name=f"qf{c}")
                           for c in range(DC)]
                for m in range(DC):
                    ps = ps_mm.tile([P, NQB], f32, tag="mm", name="qproj")
                    for (s0, sw) in _subs(nw):
                        for c in range(DC):
                            nc.tensor.matmul(
                                ps[:, s0:s0 + sw],
                                qw_sb[:, c, m * P:(m + 1) * P],
                                xa_tiles[c][:, s0:s0 + sw],
                                start=(c == 0), stop=(c == DC - 1))
                    for (s0, sw) in _subs(nw):
                        if qb_nz:
                            nc.scalar.activation(
                                q_tiles[m][:, s0:s0 + sw], ps[:, s0:s0 + sw],
                                AF.Identity, bias=qb_sb[:, m:m + 1])
                        else:
                            nc.vector.tensor_copy(q_tiles[m][:, s0:s0 + sw],
                                                  ps[:, s0:s0 + sw])
                return q_tiles

            def oproj(b, out_feat):
                """O projection of block b: per 128-token chunk, DC
                accumulating matmuls (N=768), DVE copy, DMA out."""
                n0, nw = NQ_BLOCKS[b]
                for c0 in range(0, nw, P):
                    cw = min(P, nw - c0)
                    ps = ps_mm.tile([P, NQB], f32, tag="mm", name="oproj")
                    for (s0, sw) in _subs(D):
                        for c in range(DC):
                            nc.tensor.matmul(
                                ps[:cw, s0:s0 + sw],
                                out_feat[c][:, c0:c0 + cw],
                                pw_sb[:, c, s0:s0 + sw],
                                start=(c == 0),
                                stop=(c == DC - 1 and not pb_nz))
                        if pb_nz:
                            nc.tensor.matmul(
                                ps[:cw, s0:s0 + sw], ones_bf[:, :cw],
                                pb_sb[:, s0:s0 + sw],
                                start=False, stop=True)
                    ot = otp.tile([P, D], f32, tag="ot", name="ot")
                    for (s0, sw) in _subs(D):
                        nc.vector.tensor_copy(ot[:cw, s0:s0 + sw],
                                              ps[:cw, s0:s0 + sw])
                    nc.sync.dma_start(out[n0 + c0:n0 + c0 + cw, :], ot[:cw, :])

            def finish_head(pend):
                """Off-PE softmax normalization for one finished head.

                The PV psum tile is not reused for a full head duration
                (~13us, ps_pv bufs=2), so the whole chain (denominator copy
                -> approx reciprocal -> gpsimd partition broadcast -> DVE
                multiply straight out of PSUM) runs on DVE/GpSimd without
                ever stalling the PE."""
                h, pv, pnw, out_feat = (pend["h"], pend["pv"], pend["nw"],
                                        pend["of"])
                hc, hp = h // 2, (h % 2) * HD
                den = nrm.tile([1, NQB], f32, tag="den", name="den")
                for (s0, sw) in _subs(pnw):
                    nc.vector.tensor_copy(den[:, s0:s0 + sw],
                                          pv[HD:HD + 1, s0:s0 + sw])
                rec = nrm.tile([1, NQB], f32, tag="rec", name="rec")
                nc.vector.reciprocal_approx_fast(rec[:, :pnw], den[:, :pnw])
                rbc = rbcp.tile([HD, NQB], f32, tag="rbc", name="rbc")
                nc.gpsimd.partition_broadcast(rbc[:, :pnw], rec[:, :pnw])
                for (s0, sw) in _subs(pnw):
                    nc.vector.tensor_mul(out_feat[hc][hp:hp + HD, s0:s0 + sw],
                                         pv[:HD, s0:s0 + sw],
                                         rbc[:, s0:s0 + sw])

            xa_tiles = load_xa(0)
            if n_blk > 1:
                xa_next = load_xa(1)
            q_feat = qproj(0, xa_tiles)

            # software pipeline across heads AND blocks: PV of the previous
            # head (possibly of the previous block) interleaves 1:1 with the
            # current head's score matmuls, keeping the PE gap-free at the
            # ScalarE exp pace.
            pend = None        # previous head: exp tiles + accumulating pv
            oproj_src = None   # (b, out_feat) whose o-proj is still owed

            for b, (n0, nw) in enumerate(NQ_BLOCKS):
                out_feat = [ofp.tile([P, NQB], bf16, tag=f"of{c}", name=f"of{c}")
                            for c in range(DC)]

                for h in range(H):
                    hc, hp = h // 2, (h % 2) * HD
                    exp_tiles = []
                    for ti, (t0, tw) in enumerate(TOK_CHUNKS):
                        ps = ps_mm.tile([P, NQB], f32, tag="mm", name="score")
                        for (s0, sw) in _subs(nw):
                            nc.tensor.matmul(
                                ps[:tw, s0:s0 + sw],
                                k_feat[hc][hp:hp + HD, t0:t0 + tw],
                                q_feat[hc][hp:hp + HD, s0:s0 + sw],
                                start=True, stop=True)
                        et = expp.tile([P, NQB], bf16, tag="exp", name="exp")
                        for (s0, sw) in _subs(nw):
                            nc.scalar.activation(et[:tw, s0:s0 + sw],
                                                 ps[:tw, s0:s0 + sw],
                                                 AF.Exp, scale=SCALE)
                        exp_tiles.append(et)
                        if pend is not None:
                            for (s0, sw) in _subs(pend["nw"]):
                                nc.tensor.matmul(
                                    pend["pv"][:, s0:s0 + sw],
                                    v_st[ti][:tw, pend["h"], :],
                                    pend["exp"][ti][:tw, s0:s0 + sw],
                                    start=(ti == 0), stop=(ti == NC_CH - 1))
                    if pend is not None:
                        finish_head(pend)
                        if pend["h"] == H - 1:
                            # previous block's attention fully done ->
                            # its o-proj runs as a solo PE stretch
                            oproj(oproj_src[0], oproj_src[1])
                            oproj_src = None
                    pend = {"h": h, "exp": exp_tiles, "nw": nw, "of": out_feat,
                            "pv": ps_pv.tile([HD + 1, NQB], f32, tag="pv",
                                             name="pv")}

                oproj_src = (b, out_feat)
                if b + 1 < n_blk:
                    # tail: Q-projection of the next block keeps the PE busy
                    # while the last heads' exp/PV epilogues drain
                    if b + 2 < n_blk:
                        xa_next2 = load_xa(b + 2)
                    q_feat = qproj(b + 1, xa_next)
                    xa_tiles = xa_next
                    if b + 2 < n_blk:
                        xa_next = xa_next2

            # drain: PV of the final head, then its o-proj
            for ti, (t0, tw) in enumerate(TOK_CHUNKS):
                for (s0, sw) in _subs(pend["nw"]):
                    nc.tensor.matmul(
                        pend["pv"][:, s0:s0 + sw],
                        v_st[ti][:tw, pend["h"], :],
                        pend["exp"][ti][:tw, s0:s0 + sw],
                        start=(ti == 0), stop=(ti == NC_CH - 1))
            finish_head(pend)
            oproj(oproj_src[0], oproj_src[1])

    nc.finalize()
    return nc


def kernel(**inputs) -> np.ndarray:
    import ml_dtypes
    bf = ml_dtypes.bfloat16

    s_x = np.asarray(inputs["s_x"], np.float32)
    audio = np.asarray(inputs["audio"], np.float32)
    q_w = np.asarray(inputs["q_w"], np.float32)
    q_b = np.asarray(inputs["q_b"], np.float32)
    kv_w = np.asarray(inputs["kv_w"], np.float32)
    kv_b = np.asarray(inputs["kv_b"], np.float32)
    proj_w = np.asarray(inputs["proj_w"], np.float32)
    proj_b = np.asarray(inputs["proj_b"], np.float32)

    # host prep: layout + O(N*D) positional add + bf16 casts only
    pos_s = (np.asarray(inputs["clip_space_pos"], np.float32)[:, None, :]
             + np.asarray(inputs["clip_temporal_pos"], np.float32)[None, :, :]
             ).reshape(NT, D)
    pos_a = (np.asarray(inputs["audio_space_pos"], np.float32)[:, None, :]
             + np.asarray(inputs["audio_temporal_pos"], np.float32)[None, :, :]
             ).reshape(NT, D)
    qwT = np.ascontiguousarray(q_w.T).astype(bf)
    kvwT = np.ascontiguousarray(kv_w.T).astype(bf)
    projT = np.ascontiguousarray(proj_w.T).astype(bf)
    qb_nz = bool(np.any(q_b))
    kb_nz = bool(np.any(kv_b[:D]))
    vb_nz = bool(np.any(kv_b[D:]))
    pb_nz = bool(np.any(proj_b))

    key = (qb_nz, kb_nz, vb_nz, pb_nz)
    if key not in _CACHE:
        _CACHE[key] = _build_nc(*key)
    nc = _CACHE[key]

    shared = {"qwT": qwT, "kvwT": kvwT, "projT": projT}
    if qb_nz:
        shared["qb"] = np.ascontiguousarray(q_b.reshape(DC, P).T)
    if kb_nz:
        shared["kb"] = np.ascontiguousarray(kv_b[:D].reshape(DC, P).T)
    if vb_nz:
        shared["vb"] = np.ascontiguousarray(kv_b[D:].reshape(1, D)).astype(bf)
    if pb_nz:
        shared["pb"] = np.ascontiguousarray(proj_b.reshape(1, D)).astype(bf)

    in_maps = []
    for b in range(N_CORES):
        m = dict(shared)
        m["xsT"] = np.ascontiguousarray(
            (s_x[1:, b * T:(b + 1) * T, :].reshape(NT, D) + pos_s).T).astype(bf)
        m["xaT"] = np.ascontiguousarray(
            (audio[2:, b * T:(b + 1) * T, :].reshape(NT, D) + pos_a).T).astype(bf)
        in_maps.append(m)

    from concourse.bass_utils import run_bass_kernel_spmd
    res = run_bass_kernel_spmd(nc, in_maps, core_ids=list(range(N_CORES)))
    globals()["_LAST_RESULT"] = res

    out_full = np.empty((2 + APATCH, B * T, D), np.float32)
    out_full[:2] = audio[:2]
    for b in range(N_CORES):
        out_full[2:, b * T:(b + 1) * T, :] = \
            res.results[b]["out"].reshape(APATCH, T, D)
    return out_full
